# revision 7
# baseline (speedup 1.0000x reference)
"""Trainium2 Bass kernel for the CLC block (grouped 3x3 conv -> BN+ReLU ->
grouped 1x1 conv -> BN+ReLU, twice).

Sharding: pure data parallel, batch 32 -> 4 samples per core on 8 cores.

Per-core design (all f32 storage, float32r matmul views):
  - Channel-major layout: [128 channel partitions, pixels] per 128-channel half.
  - gconv3x3: the torch concat ordering (out o = i*64+g reads inputs 4g..4g+3)
    is made block-diagonal per half by storing gconv OUTPUTS in "g-major"
    order (pos = 4g+i) while gconv INPUTS stay in natural channel order.
    Each tap (dh,dw) is a [K=128,M=128] matmul whose rhs is a shifted window
    of a zero-padded [128, 58*58] input tile; 9 taps accumulate in PSUM.
  - pw 1x1 conv: contracts a full 64-channel block which spans both halves of
    the g-major layout -> 2 accumulating K=128 matmuls per output half.
  - BN + conv-bias fold into the matmul weights (host side); each stage then
    needs a single bias+ReLU pass evacuating PSUM->SBUF (ACT for half 0,
    DVE tensor_scalar add+max for half 1).
"""

import numpy as np

B, C, H, W = 32, 256, 56, 56
EPS = 1e-5
N_CORES = 8
BPC = B // N_CORES  # samples per core
HP, WP = H + 2, W + 2  # padded spatial
NPIX = H * W
NPAD = HP * WP
ROWS_PER_TILE = 8
NT = H // ROWS_PER_TILE  # 7 pixel tiles
TILE_PX = ROWS_PER_TILE * W  # 448


# ---------------------------------------------------------------------------
# Host-side weight preparation
# ---------------------------------------------------------------------------

def _bn_fold(bg, bb, bm, bv):
    inv = bg / np.sqrt(bv + EPS)
    return inv, bb - bm * inv  # scale, shift (applied after conv+bias*scale)


def prepare_weights(inp):
    """Returns (wg [128, 2*2*9*128], wp [128, 2*2*2*128], bias [128, 8]) f32.

    wg[k, ((l*2+h)*9 + t)*128 + m]: lhsT for gconv layer l, output half h,
      tap t=3*dh+dw.  k = natural input channel within half h; m = g-major
      output position (g = 32h + m//4, i = m%4).
    wp[k, ((l*2+H)*2 + A)*128 + m]: lhsT for pw layer l, output half H
      (natural order), input half A of the g-major input layout.
    bias[m, s*2 + h]: per-partition bias for stage s in that stage's output
      layout (s=0,2: g-major; s=1,3: natural).
    """
    f32 = np.float32
    wg = np.zeros((128, 2, 2, 9, 128), f32)
    wp = np.zeros((128, 2, 2, 2, 128), f32)
    bias = np.zeros((128, 8), f32)

    gconv_params = [
        (inp["w1"], inp["b1"], inp["bn1a_g"], inp["bn1a_b"], inp["bn1a_m"], inp["bn1a_v"]),
        (inp["w2"], inp["b2"], inp["bn2a_g"], inp["bn2a_b"], inp["bn2a_m"], inp["bn2a_v"]),
    ]
    pw_params = [
        (inp["pw1"], inp["pb1"], inp["bn1b_g"], inp["bn1b_b"], inp["bn1b_m"], inp["bn1b_v"]),
        (inp["pw2"], inp["pb2"], inp["bn2b_g"], inp["bn2b_b"], inp["bn2b_m"], inp["bn2b_v"]),
    ]

    for l, (w, bcv, bg, bb, bm, bv) in enumerate(gconv_params):
        w = np.asarray(w, f32)
        inv, shift = _bn_fold(np.asarray(bg, f32), np.asarray(bb, f32),
                              np.asarray(bm, f32), np.asarray(bv, f32))
        bconv = np.asarray(bcv, f32).reshape(256)  # index i*64+g
        beff = bconv * inv + shift  # natural order o
        for h in range(2):
            for m in range(128):
                g = 32 * h + m // 4
                i = m % 4
                o = i * 64 + g
                for kk in range(4):
                    k = 4 * g + kk - 128 * h
                    for t in range(9):
                        wg[k, l, h, t, m] = w[i, g, kk, t // 3, t % 3] * inv[o]
                bias[m, (2 * l) * 2 + h] = beff[o]

    for l, (w, pb, bg, bb, bm, bv) in enumerate(pw_params):
        w = np.asarray(w, f32).reshape(256, 64)
        inv, shift = _bn_fold(np.asarray(bg, f32), np.asarray(bb, f32),
                              np.asarray(bm, f32), np.asarray(bv, f32))
        beff = np.asarray(pb, f32) * inv + shift
        for Hh in range(2):
            for m in range(128):
                c = 128 * Hh + m
                i = c // 64
                for kap in range(64):
                    p = 4 * kap + i  # g-major position of input channel 64*i+kap
                    A, k = divmod(p, 128)
                    wp[k, l, Hh, A, m] = w[c, kap] * inv[c]
                bias[m, (2 * l + 1) * 2 + Hh] = beff[c]

    return (wg.reshape(128, 2 * 2 * 9 * 128),
            wp.reshape(128, 2 * 2 * 2 * 128),
            bias)


# ---------------------------------------------------------------------------
# Numpy emulation of the exact kernel dataflow (for validation)
# ---------------------------------------------------------------------------

def emulate(inp):
    wg, wp, bias = prepare_weights(inp)
    wg = wg.reshape(128, 2, 2, 9, 128)
    wp = wp.reshape(128, 2, 2, 2, 128)
    x = np.asarray(inp["x"], np.float32)  # [B, 256, 56, 56]
    out = np.zeros_like(x)

    for n in range(B):
        # natural-order padded input [2][128, 58, 58]
        xpad = np.zeros((2, 128, HP, WP), np.float32)
        for h in range(2):
            xpad[h, :, 1:57, 1:57] = x[n, 128 * h:128 * (h + 1)]

        def gconv(src_pad, l):
            t = [np.zeros((128, H, W), np.float32) for _ in range(2)]
            for h in range(2):
                acc = np.zeros((128, H, W), np.float32)
                for tap in range(9):
                    dh, dw = tap // 3, tap % 3
                    rhs = src_pad[h][:, dh:dh + H, dw:dw + W].reshape(128, -1)
                    acc += (wg[:, l, h, tap, :].T @ rhs).reshape(128, H, W)
                t[h] = np.maximum(acc + bias[:, (2 * l) * 2 + h][:, None, None], 0.0)
            return t  # g-major dense halves

        def pw(tsrc, l):
            dst = [None, None]
            for Hh in range(2):
                acc = np.zeros((128, H * W), np.float32)
                for A in range(2):
                    acc += wp[:, l, Hh, A, :].T @ tsrc[A].reshape(128, -1)
                r = np.maximum(acc + bias[:, (2 * l + 1) * 2 + Hh][:, None], 0.0)
                dst[Hh] = r.reshape(128, H, W)
            return dst  # natural dense halves

        t1 = gconv(xpad, 0)
        t2 = pw(t1, 0)
        t2pad = np.zeros((2, 128, HP, WP), np.float32)
        for h in range(2):
            t2pad[h, :, 1:57, 1:57] = t2[h]
        t3 = gconv(t2pad, 1)
        y = pw(t3, 1)
        out[n, 0:128] = y[0]
        out[n, 128:256] = y[1]
    return out


# ---------------------------------------------------------------------------
# Bass program
# ---------------------------------------------------------------------------

_CACHED = {}
_REPEAT = [1]


def set_repeat(r):
    _REPEAT[0] = r


def _build_body(tc, y_ap, x_ap, wg_ap, wp_ap, bias_ap, zeros_ap, repeat=1):
    import concourse.bass as bass  # noqa: F401
    from concourse import mybir

    nc = tc.nc
    f32 = mybir.dt.float32
    f32r = mybir.dt.float32r
    ADD = mybir.AluOpType.add
    MAX = mybir.AluOpType.max
    RELU = mybir.ActivationFunctionType.Relu

    import contextlib
    ctx = tc._build_ctx  # ExitStack supplied by caller

    const = ctx.enter_context(tc.tile_pool(name="const", bufs=1))
    persist = ctx.enter_context(tc.tile_pool(name="persist", bufs=1))
    gps = ctx.enter_context(tc.tile_pool(name="gps", bufs=3, space="PSUM"))
    pps = ctx.enter_context(tc.tile_pool(name="pps", bufs=3, space="PSUM"))

    wg_sb = const.tile([128, 2 * 2 * 9 * 128], f32r, tag="wg", name="wg_sb")
    wp_sb = const.tile([128, 2 * 2 * 2 * 128], f32r, tag="wp", name="wp_sb")
    bias_sb = const.tile([128, 8], f32, tag="bias", name="bias_sb")
    nc.sync.dma_start(wg_sb[:], wg_ap)
    nc.sync.dma_start(wp_sb[:], wp_ap)
    nc.sync.dma_start(bias_sb[:], bias_ap)

    # padded-layout input tiles (borders stay zero forever)
    xpad = [persist.tile([128, NPAD], f32r, tag=f"xpad{h}", name=f"xpad{h}") for h in range(2)]
    r2pad = [persist.tile([128, NPAD], f32r, tag=f"r2pad{h}", name=f"r2pad{h}") for h in range(2)]
    # dense intermediates (t1 reused for t3)
    td = [persist.tile([128, NPIX], f32r, tag=f"td{h}", name=f"td{h}") for h in range(2)]
    ysb = [persist.tile([128, NPIX], f32, tag=f"ysb{h}", name=f"ysb{h}") for h in range(2)]

    def p3(tile_):  # [128, NPAD] -> [128, 58, 58]
        return tile_[:].rearrange("p (a b) -> p a b", b=WP)

    for t in xpad + r2pad:
        v = p3(t)
        flat = t[:]
        nc.sync.dma_start(flat[:, 0:WP], zeros_ap[:, 0:WP])
        nc.sync.dma_start(flat[:, (HP - 1) * WP:HP * WP], zeros_ap[:, 0:WP])
        nc.sync.dma_start(v[:, 1:HP - 1, 0:1], zeros_ap[:, 0:HP - 2])
        nc.sync.dma_start(v[:, 1:HP - 1, WP - 1:WP], zeros_ap[:, 0:HP - 2])

    def relu_pass(dst, ps, scol, h):
        # dst = relu(psum + bias[:, scol]) ; ACT on half 0, DVE on half 1
        if h == 0:
            nc.scalar.activation(dst, ps, RELU, bias=bias_sb[:, scol:scol + 1])
        else:
            nc.vector.tensor_scalar(dst, ps, bias_sb[:, scol:scol + 1], 0.0,
                                    op0=ADD, op1=MAX)

    def gconv_stage(src_pads, dst_halves, l):
        for h in range(2):
            src = p3(src_pads[h])
            for pt in range(NT):
                ps = gps.tile([128, TILE_PX], f32, tag="g", name="psg")
                r0 = pt * ROWS_PER_TILE
                for tap in range(9):
                    dh, dw = tap // 3, tap % 3
                    rhs = src[:, r0 + dh:r0 + dh + ROWS_PER_TILE, dw:dw + W]
                    lhsT = wg_sb[:, ((l * 2 + h) * 9 + tap) * 128:
                                 ((l * 2 + h) * 9 + tap) * 128 + 128]
                    nc.tensor.matmul(ps[:], lhsT=lhsT, rhs=rhs,
                                     start=(tap == 0), stop=(tap == 8))
                dst = dst_halves[h][:, r0 * W:r0 * W + TILE_PX]
                relu_pass(dst, ps[:], (2 * l) * 2 + h, h)

    def pw_stage(src_halves, dst_fn, l):
        for Hh in range(2):
            for pt in range(NT):
                ps = pps.tile([128, TILE_PX], f32, tag="p", name="psp")
                for A in range(2):
                    lhsT = wp_sb[:, ((l * 2 + Hh) * 2 + A) * 128:
                                 ((l * 2 + Hh) * 2 + A) * 128 + 128]
                    rhs = src_halves[A][:, pt * TILE_PX:(pt + 1) * TILE_PX]
                    nc.tensor.matmul(ps[:], lhsT=lhsT, rhs=rhs,
                                     start=(A == 0), stop=(A == 1))
                dst = dst_fn(Hh, pt)
                relu_pass(dst, ps[:], (2 * l + 1) * 2 + Hh, Hh)

    for rep in range(repeat):
      for n in range(BPC):
        for h in range(2):
            src = x_ap[n, 128 * h:128 * (h + 1), :].rearrange(
                "p (a b) -> p a b", b=W)
            nc.sync.dma_start(p3(xpad[h])[:, 1:57, 1:57], src)

        gconv_stage(xpad, td, 0)

        def r2_dst(Hh, pt):
            return p3(r2pad[Hh])[:, pt * ROWS_PER_TILE + 1:
                                 pt * ROWS_PER_TILE + 1 + ROWS_PER_TILE, 1:57]
        pw_stage(td, r2_dst, 0)

        gconv_stage(r2pad, td, 1)

        def y_dst(Hh, pt):
            return ysb[Hh][:, pt * TILE_PX:(pt + 1) * TILE_PX]
        pw_stage(td, y_dst, 1)

        for h in range(2):
            dst = y_ap[n, 128 * h:128 * (h + 1), :]
            nc.sync.dma_start(dst, ysb[h][:])


def build_program(repeat=1):
    import contextlib

    import concourse.tile as tile
    from concourse import bacc, mybir

    f32 = mybir.dt.float32
    nc = bacc.Bacc("TRN2", target_bir_lowering=False, debug=False,
                   num_devices=N_CORES)
    f32r = mybir.dt.float32r
    x_d = nc.dram_tensor("x", [BPC, C, NPIX], f32r, kind="ExternalInput").ap()
    wg_d = nc.dram_tensor("wg", [128, 2 * 2 * 9 * 128], f32r,
                          kind="ExternalInput").ap()
    wp_d = nc.dram_tensor("wp", [128, 2 * 2 * 2 * 128], f32r,
                          kind="ExternalInput").ap()
    bias_d = nc.dram_tensor("bias", [128, 8], f32, kind="ExternalInput").ap()
    zeros_d = nc.dram_tensor("zeros", [128, 64], f32r, kind="ExternalInput").ap()
    y_d = nc.dram_tensor("y", [BPC, C, NPIX], f32, kind="ExternalOutput").ap()

    with tile.TileContext(nc) as tc:
        with contextlib.ExitStack() as ctx:
            tc._build_ctx = ctx
            _build_body(tc, y_d, x_d, wg_d, wp_d, bias_d, zeros_d, repeat=repeat)
    nc.compile()
    return nc


def _run(inputs, trace=False):
    from concourse.bass_utils import run_bass_kernel_spmd

    wg, wp, bias = prepare_weights(inputs)
    x = np.ascontiguousarray(np.asarray(inputs["x"], np.float32))

    key = ("nc", _REPEAT[0])
    if key not in _CACHED:
        _CACHED[key] = build_program(repeat=_REPEAT[0])
    nc = _CACHED[key]

    in_maps = []
    for i in range(N_CORES):
        in_maps.append({
            "x": x[i * BPC:(i + 1) * BPC].reshape(BPC, C, NPIX),
            "wg": wg, "wp": wp, "bias": bias,
            "zeros": np.zeros((128, 64), np.float32),
        })
    res = run_bass_kernel_spmd(nc, in_maps, list(range(N_CORES)), trace=trace)
    out = np.concatenate(
        [res.results[i]["y"].reshape(BPC, C, H, W) for i in range(N_CORES)],
        axis=0)
    return out, res


def kernel(**inputs):
    return _run(inputs)[0]
